# revision 11
# baseline (speedup 1.0000x reference)
"""Trainium2 Bass kernel for nn_Decoder: FCN -> 256-step GRU scan -> projection.

Strategy: data-parallel over batch (256 -> 32 per core, 8 cores, no cross-core
communication). Per step the GRU matmul h @ Wc.T uses h.T tiles as the
stationary operand (batch=32 on the stationary free dim) and the combined
gate weights as the bf16 moving operand, with 4-way PE column tiling: column
group g computes the gate columns [r|z|in|hn] of H-chunk c, so the 4 partial
psums stack on partitions (4 groups x 32 batch = 128) and every gate
elementwise op runs on the full 128 partitions with aligned operands.
r/z gates are algebraically folded (input==hidden) so the per-step matmul is
[32,1024]@[1024,4096] instead of @[1024,6144].
"""

import numpy as np
import ml_dtypes

import concourse.bass as bass
import concourse.mybir as mybir
import concourse.tile as tile
from concourse.bass_utils import run_bass_kernel_spmd

F32 = mybir.dt.float32
F32R = mybir.dt.float32r
BF16 = mybir.dt.bfloat16
ACTF = mybir.ActivationFunctionType

B, D_IN, H, D_OUT = 256, 512, 1024, 512
NC = 8           # cores
BC = B // NC     # batch per core = 32
KT = H // 128    # K tiles = 8


def _round_fp32r(a):
    b = np.ascontiguousarray(a, np.float32).view(np.uint32).astype(np.uint64)
    lsb = (b >> 12) & 1
    r = (b + 0x7FF + lsb) & np.uint64(0xFFFFF000)
    return r.astype(np.uint32).view(np.float32)


def _bf16(a):
    return np.ascontiguousarray(a, np.float32).astype(ml_dtypes.bfloat16)


def fix_multiwaits(nc):
    """Split multi-wait instructions into single-wait NOPs + instruction.

    The TRN2 codegen here accepts at most one sync-wait slot per instruction
    (and none reliably on Matmult); Tile attaches all waits to the consumer.
    """
    n_split = 0
    for fn in nc.m.functions:
        for bb in fn.blocks:
            new_insts = []
            for inst in bb.instructions:
                si = inst.sync_info
                waits = list(si.on_wait) if (si and si.on_wait) else []
                keep = 0 if type(inst).__name__ == "InstMatmult" else 1
                if len(waits) > keep:
                    moved = waits[:len(waits) - keep]
                    kept = waits[len(waits) - keep:]
                    for w in moved:
                        n_split += 1
                        nop = mybir.InstNoOp(
                            name=f"I-waitsplit-{n_split}",
                            engine=inst.engine,
                            ins=[], outs=[],
                            sync_info=mybir.SyncInfo(on_wait=[w], on_update=[]),
                        )
                        new_insts.append(nop)
                    inst.sync_info = mybir.SyncInfo(
                        on_wait=kept,
                        on_update=list(si.on_update) if si else [])
                new_insts.append(inst)
            bb.instructions[:] = new_insts
    return n_split


def build(T, fix=True, do_scan=True, do_proj=True, do_hst=True, do_bias=True, part='full'):
    nc = bass.Bass()

    # -------- inputs (per core; weights identical across cores) --------
    xT_d = nc.declare_dram_parameter("xT", [128, 4 * BC], F32R, isOutput=False)
    w1t_d = nc.declare_dram_parameter("w1t", [128, 4 * H], F32R, isOutput=False)
    w2t_d = nc.declare_dram_parameter("w2t", [128, 8 * H], F32R, isOutput=False)
    b1r_d = nc.declare_dram_parameter("b1r", [1, H], F32R, isOutput=False)
    b2r_d = nc.declare_dram_parameter("b2r", [1, H], F32R, isOutput=False)
    wq_d = nc.declare_dram_parameter("wq", [128, 64 * 512], BF16, isOutput=False)
    biasq_d = nc.declare_dram_parameter("biasq", [1, 8 * 512], BF16, isOutput=False)
    owt_d = nc.declare_dram_parameter("owt", [128, 8 * D_OUT], BF16, isOutput=False)
    obr_d = nc.declare_dram_parameter("obr", [1, D_OUT], BF16, isOutput=False)
    onesr_d = nc.declare_dram_parameter("onesr", [1, BC], F32R, isOutput=False)
    onesb_d = nc.declare_dram_parameter("onesb", [1, 128], BF16, isOutput=False)
    ident_d = nc.declare_dram_parameter("ident", [128, 32], F32, isOutput=False)

    recon_d = nc.declare_dram_parameter("recon", [BC, T, D_OUT], F32, isOutput=True)
    lat_d = nc.declare_dram_parameter("latent", [BC, H], F32, isOutput=True)

    lat_tmp = nc.dram_tensor("lat_tmp", [BC, H], F32)
    hsT_d = nc.dram_tensor("hsT", [128, KT * T * BC], BF16)

    with tile.TileContext(nc) as tc:
        with (
            tc.tile_pool(name="const", bufs=1) as cpool,
            tc.tile_pool(name="work", bufs=2) as wpool,
            tc.tile_pool(name="proj", bufs=3) as ppool,
            tc.tile_pool(name="ps", bufs=2, space="PSUM") as ps,
            tc.tile_pool(name="psx", bufs=1, space="PSUM") as psx,
        ):
            # ---------------- load constants ----------------
            xT = cpool.tile([128, 4 * BC], F32R, tag="xT")
            nc.sync.dma_start(out=xT[:], in_=xT_d[:, :])
            w1t = cpool.tile([128, 4 * H], F32R, tag="w1t")
            nc.sync.dma_start(out=w1t[:], in_=w1t_d[:, :])
            w2t = cpool.tile([128, 8 * H], F32R, tag="w2t")
            nc.sync.dma_start(out=w2t[:], in_=w2t_d[:, :])
            b1r = cpool.tile([1, H], F32R, tag="b1r")
            nc.sync.dma_start(out=b1r[:], in_=b1r_d[:, :])
            b2r = cpool.tile([1, H], F32R, tag="b2r")
            nc.sync.dma_start(out=b2r[:], in_=b2r_d[:, :])
            wq = cpool.tile([128, 64 * 512], BF16, tag="wq")
            nc.sync.dma_start(out=wq[:], in_=wq_d[:, :])
            biasq = cpool.tile([1, 8 * 512], BF16, tag="biasq")
            nc.sync.dma_start(out=biasq[:], in_=biasq_d[:, :])
            owt = cpool.tile([128, 8 * D_OUT], BF16, tag="owt")
            nc.sync.dma_start(out=owt[:], in_=owt_d[:, :])
            obr = cpool.tile([1, D_OUT], BF16, tag="obr")
            nc.sync.dma_start(out=obr[:], in_=obr_d[:, :])
            onesr = cpool.tile([1, BC], F32R, tag="onesr")
            nc.sync.dma_start(out=onesr[:], in_=onesr_d[:, :])
            onesb = cpool.tile([1, 128], BF16, tag="onesb")
            nc.sync.dma_start(out=onesb[:], in_=onesb_d[:, :])
            ident = cpool.tile([128, 32], F32, tag="ident")
            nc.sync.dma_start(out=ident[:], in_=ident_d[:, :])

            # ---------------- FCN (fp32r) ----------------
            # l1 = x @ w1.T + b1  -> [32, 1024] as 2 psum tiles of [32, 512]
            l1sb = wpool.tile([BC, H], F32, tag="l1sb")
            for n in range(2):
                pl = ps.tile([BC, 512], F32, tag="pq0")
                nc.tensor.matmul(pl[:], onesr[:], b1r[0:1, n * 512:(n + 1) * 512],
                                 start=True, stop=False)
                for k in range(4):
                    nc.tensor.matmul(
                        pl[:], xT[:, k * BC:(k + 1) * BC],
                        w1t[:, k * H + n * 512:k * H + (n + 1) * 512],
                        start=False, stop=(k == 3))
                nc.scalar.copy(l1sb[:, n * 512:(n + 1) * 512], pl[:])
            # transpose l1 -> l1T [128, 8*32] fp32r
            l1T = wpool.tile([128, KT * BC], F32R, tag="l1T")
            for k in range(KT):
                pt2 = psx.tile([128, BC], F32, tag=f"pt{k % 4}")
                nc.tensor.transpose(pt2[:], l1sb[:, k * 128:(k + 1) * 128],
                                    ident[0:32, :])
                nc.scalar.copy(l1T[:, k * BC:(k + 1) * BC], pt2[:])
            # l2 = l1 @ w2.T + b2 -> latent [32, 1024]
            latsb = wpool.tile([BC, H], F32, tag="latsb")
            for n in range(2):
                pl2 = ps.tile([BC, 512], F32, tag="pq0")
                nc.tensor.matmul(pl2[:], onesr[:], b2r[0:1, n * 512:(n + 1) * 512],
                                 start=True, stop=False)
                for k in range(KT):
                    nc.tensor.matmul(
                        pl2[:], l1T[:, k * BC:(k + 1) * BC],
                        w2t[:, k * H + n * 512:k * H + (n + 1) * 512],
                        start=False, stop=(k == KT - 1))
                nc.scalar.copy(latsb[:, n * 512:(n + 1) * 512], pl2[:])
            nc.sync.dma_start(out=lat_d[:, :], in_=latsb[:])
            nc.sync.dma_start(out=lat_tmp[:, :], in_=latsb[:])
            # initial state in both layouts via DRAM roundtrip
            hq0 = wpool.tile([128, 256], F32, tag="hq")
            for g in range(4):
                nc.sync.dma_start(
                    out=hq0[g * 32:(g + 1) * 32, :].rearrange(
                        "b (q j) -> b q j", q=2),
                    in_=lat_tmp[:].rearrange(
                        "b (q g j) -> g b q j", q=2, g=4)[g])
            hT0 = wpool.tile([128, KT * BC], BF16, tag="hT")
            for k in range(KT):
                nc.gpsimd.dma_start(
                    out=hT0[:, k * BC:(k + 1) * BC],
                    in_=lat_tmp[:, k * 128:(k + 1) * 128].rearrange("b p -> p b"))

            # ---------------- GRU scan ----------------
            hq_prev, hT_prev = hq0, hT0
            for t in range(T if do_scan else 0):
                hq_next = wpool.tile([128, 256], F32, tag="hq")
                hT_next = wpool.tile([128, KT * BC], BF16, tag="hT")
                for q in range(2):
                    pq = ps.tile([128, 512], F32, tag=f"pq{q}")
                    # bias row for each col group, then K-major interleave
                    if do_bias:
                        for g in range(4):
                            c = 4 * q + g
                            nc.tensor.matmul(
                                pq[g * 32:(g + 1) * 32, :],
                                onesb[0:1, 0:BC],
                                biasq[0:1, c * 512:(c + 1) * 512],
                                start=True, stop=False,
                                tile_position=(0, g * 32),
                                skip_group_check=True)
                    for k in range(KT):
                        for g in range(4):
                            c = 4 * q + g
                            nc.tensor.matmul(
                                pq[g * 32:(g + 1) * 32, :],
                                hT_prev[:, k * BC:(k + 1) * BC],
                                wq[:, (k * 8 + c) * 512:(k * 8 + c + 1) * 512],
                                start=(not do_bias and k == 0),
                                stop=(k == KT - 1),
                                tile_position=(0, g * 32),
                                skip_group_check=True)
                    if part == "mm":
                        dump = wpool.tile([128, 512], F32, tag=f"dump{q}")
                        nc.vector.tensor_copy(dump[:], pq[:])
                        continue
                    # gates: pq cols = [r 0:128 | z 128:256 | in 256:384 | hn 384:512]
                    rz = wpool.tile([128, 256], F32, tag=f"rz{q}")
                    nc.scalar.activation(rz[:], pq[:, 0:256], ACTF.Sigmoid)
                    t1 = wpool.tile([128, 128], F32, tag=f"t1{q}")
                    nc.vector.tensor_mul(t1[:], rz[:, 0:128], pq[:, 384:512])
                    t2 = wpool.tile([128, 128], F32, tag=f"t2{q}")
                    nc.vector.tensor_add(t2[:], t1[:], pq[:, 256:384])
                    nt = wpool.tile([128, 128], F32, tag=f"nt{q}")
                    nc.scalar.activation(nt[:], t2[:], ACTF.Tanh)
                    t3 = wpool.tile([128, 128], F32, tag=f"t3{q}")
                    nc.vector.tensor_sub(t3[:], hq_prev[:, q * 128:(q + 1) * 128],
                                         nt[:])
                    t4 = wpool.tile([128, 128], F32, tag=f"t4{q}")
                    nc.vector.tensor_mul(t4[:], rz[:, 128:256], t3[:])
                    nc.vector.tensor_add(hq_next[:, q * 128:(q + 1) * 128],
                                         nt[:], t4[:])
                    if part == "mmg":
                        continue
                    # transposes: one psum bank per transpose (HW constraint)
                    for g in range(4):
                        c = 4 * q + g
                        ptg = psx.tile([128, BC], F32, tag=f"pt{g}")
                        nc.tensor.transpose(
                            ptg[:],
                            hq_next[g * 32:(g + 1) * 32, q * 128:(q + 1) * 128],
                            ident[g * 32:(g + 1) * 32, :],
                            tile_position=(g * 32, 0))
                        eng = nc.scalar if g % 2 == 0 else nc.vector
                        if g % 2 == 0:
                            nc.scalar.copy(hT_next[:, c * BC:(c + 1) * BC],
                                           ptg[:])
                        else:
                            nc.vector.tensor_copy(hT_next[:, c * BC:(c + 1) * BC],
                                                  ptg[:])
                if do_hst:
                    nc.sync.dma_start(
                        out=hsT_d[:].rearrange("p (k tb) -> p k tb", k=KT)
                        [:, :, t * BC:(t + 1) * BC],
                        in_=hT_next[:].rearrange("p (k b) -> p k b", k=KT))
                if part == "full":
                    hq_prev, hT_prev = hq_next, hT_next
                elif part == "mmg":
                    hq_prev = hq_next

            # ---------------- output projection (bf16) ----------------
            n_chunks = (T * BC) // 128 if (do_scan and do_proj) else 0
            for i in range(n_chunks):
                ch = ppool.tile([128, KT * 128], BF16, tag="proj_ch")
                nc.sync.dma_start(
                    out=ch[:].rearrange("p (k tb) -> p k tb", k=KT),
                    in_=hsT_d[:].rearrange("p (k tb) -> p k tb", k=KT)
                    [:, :, i * 128:(i + 1) * 128])
                pp = ps.tile([128, D_OUT], F32, tag="pq1")
                nc.tensor.matmul(pp[:], onesb[:], obr[:],
                                 start=True, stop=False)
                for k in range(KT):
                    nc.tensor.matmul(
                        pp[:], ch[:, k * 128:(k + 1) * 128],
                        owt[:, k * D_OUT:(k + 1) * D_OUT],
                        start=False, stop=(k == KT - 1))
                outc = ppool.tile([128, D_OUT], F32, tag="outc")
                if i % 2 == 0:
                    nc.scalar.copy(outc[:], pp[:])
                else:
                    nc.vector.tensor_copy(outc[:], pp[:])
                nc.sync.dma_start(
                    out=recon_d[0:BC, 4 * i:4 * i + 4, :]
                    .rearrange("b t d -> t b d"),
                    in_=outc[:])

    if fix:
        fix_multiwaits(nc)
    return nc


_built = {}


def _get_nc(T):
    import os
    key = (T, os.environ.get("K_SCAN", "1"), os.environ.get("K_PROJ", "1"),
           os.environ.get("K_FIX", "1"), os.environ.get("K_HST", "1"),
           os.environ.get("K_BIAS", "1"), os.environ.get("K_PART", "full"))
    if key not in _built:
        _built[key] = build(T, fix=key[3] == "1", do_scan=key[1] == "1",
                            do_proj=key[2] == "1", do_hst=key[4] == "1",
                            do_bias=key[5] == "1", part=key[6])
    return _built[key]


def _prepare_maps(x, fcn_w1, fcn_b1, fcn_w2, fcn_b2, w_ih, b_ih, w_hh, b_hh,
                  out_w, out_b):
    # FCN weights (fp32r), k-major partition layout [128, K*cols]
    w1t = _round_fp32r(
        fcn_w1.T.reshape(4, 128, H).transpose(1, 0, 2).reshape(128, 4 * H))
    w2t = _round_fp32r(
        fcn_w2.T.reshape(8, 128, H).transpose(1, 0, 2).reshape(128, 8 * H))
    b1r = _round_fp32r(fcn_b1.reshape(1, H))
    b2r = _round_fp32r(fcn_b2.reshape(1, H))

    # combined gate weights: Wc rows = [r(1024) z(1024) in(1024) hn(1024)]
    Wc = np.concatenate([w_ih[:2 * H] + w_hh[:2 * H], w_ih[2 * H:],
                         w_hh[2 * H:]], axis=0)          # [4H, H]
    biasc = np.concatenate([b_ih[:2 * H] + b_hh[:2 * H], b_ih[2 * H:],
                            b_hh[2 * H:]])               # [4H]
    # wq[p, (k*8+c)*512 + gate*128 + j] = Wc[gate*1024 + c*128 + j, k*128+p]
    wq = _bf16(Wc.reshape(4, 8, 128, 8, 128)            # [gate, c, j, k, p]
               .transpose(4, 3, 1, 0, 2)                # [p, k, c, gate, j]
               .reshape(128, 64 * 512))
    # biasq[0, c*512 + gate*128 + j] = biasc[gate*1024 + c*128 + j]
    biasq = _bf16(biasc.reshape(4, 8, 128)              # [gate, c, j]
                  .transpose(1, 0, 2)                   # [c, gate, j]
                  .reshape(1, 8 * 512))
    owt = _bf16(out_w.T.reshape(8, 128, D_OUT)
                .transpose(1, 0, 2).reshape(128, 8 * D_OUT))
    obr = _bf16(out_b.reshape(1, D_OUT))
    onesr = _round_fp32r(np.ones((1, BC), np.float32))
    onesb = _bf16(np.ones((1, 128), np.float32))
    ident = np.zeros((128, 32), np.float32)
    for g in range(4):
        ident[g * 32:(g + 1) * 32, :] = np.eye(32, dtype=np.float32)

    maps = []
    for c in range(NC):
        xc = x[c * BC:(c + 1) * BC]                     # [32, 512]
        xT = _round_fp32r(
            xc.T.reshape(4, 128, BC).transpose(1, 0, 2).reshape(128, 4 * BC))
        maps.append({
            "xT": xT, "w1t": w1t, "w2t": w2t, "b1r": b1r, "b2r": b2r,
            "wq": wq, "biasq": biasq, "owt": owt, "obr": obr,
            "onesr": onesr, "onesb": onesb, "ident": ident,
        })
    return maps


def kernel(x, fcn_w1, fcn_b1, fcn_w2, fcn_b2, w_ih, b_ih, w_hh, b_hh,
           out_w, out_b, n_steps, _trace=False, _trace_kwargs=None):
    T = int(n_steps)
    x = np.ascontiguousarray(x, np.float32)
    args = [np.ascontiguousarray(np.asarray(a), np.float32)
            for a in (fcn_w1, fcn_b1, fcn_w2, fcn_b2, w_ih, b_ih, w_hh, b_hh,
                      out_w, out_b)]
    nc = _get_nc(T)
    maps = _prepare_maps(x, *args)
    kw = {}
    if _trace:
        kw = {"trace": True}
        if _trace_kwargs:
            kw.update(_trace_kwargs)
    res = run_bass_kernel_spmd(nc, maps, list(range(NC)), **kw)
    recon = np.concatenate([res.results[c]["recon"] for c in range(NC)], axis=0)
    lat = np.concatenate([res.results[c]["latent"] for c in range(NC)], axis=0)
    kernel._last_results = res
    return recon, lat


# revision 15
# speedup vs baseline: 1.0026x; 1.0026x over previous
"""Trainium2 Bass kernel for nn_Decoder: FCN -> 256-step GRU scan -> projection.

Strategy: data-parallel over batch (256 -> 32 per core, 8 cores, no cross-core
communication). Per step the GRU matmul h @ Wc.T uses h.T tiles as the
stationary operand (batch=32 on the stationary free dim) and the combined
gate weights as the bf16 moving operand, with 4-way PE column tiling: column
group g computes the gate columns [r|z|in|hn] of H-chunk c, so the 4 partial
psums stack on partitions (4 groups x 32 batch = 128) and every gate
elementwise op runs on the full 128 partitions with aligned operands.
r/z gates are algebraically folded (input==hidden) so the per-step matmul is
[32,1024]@[1024,4096] instead of @[1024,6144].
"""

import numpy as np
import ml_dtypes

import concourse.bass as bass
import concourse.mybir as mybir
import concourse.tile as tile
from concourse.bass_utils import run_bass_kernel_spmd

F32 = mybir.dt.float32
F32R = mybir.dt.float32r
BF16 = mybir.dt.bfloat16
ACTF = mybir.ActivationFunctionType

B, D_IN, H, D_OUT = 256, 512, 1024, 512
NC = 8           # cores
BC = B // NC     # batch per core = 32
KT = H // 128    # K tiles = 8


def _round_fp32r(a):
    b = np.ascontiguousarray(a, np.float32).view(np.uint32).astype(np.uint64)
    lsb = (b >> 12) & 1
    r = (b + 0x7FF + lsb) & np.uint64(0xFFFFF000)
    return r.astype(np.uint32).view(np.float32)


def _bf16(a):
    return np.ascontiguousarray(a, np.float32).astype(ml_dtypes.bfloat16)


def fix_multiwaits(nc):
    """Split multi-wait instructions into single-wait NOPs + instruction.

    The TRN2 codegen here accepts at most one sync-wait slot per instruction
    (and none reliably on Matmult); Tile attaches all waits to the consumer.
    """
    n_split = 0
    for fn in nc.m.functions:
        for bb in fn.blocks:
            new_insts = []
            for inst in bb.instructions:
                si = inst.sync_info
                waits = list(si.on_wait) if (si and si.on_wait) else []
                keep = 0 if type(inst).__name__ == "InstMatmult" else 1
                if len(waits) > keep:
                    moved = waits[:len(waits) - keep]
                    kept = waits[len(waits) - keep:]
                    for w in moved:
                        n_split += 1
                        nop = mybir.InstNoOp(
                            name=f"I-waitsplit-{n_split}",
                            engine=inst.engine,
                            ins=[], outs=[],
                            sync_info=mybir.SyncInfo(on_wait=[w], on_update=[]),
                        )
                        new_insts.append(nop)
                    inst.sync_info = mybir.SyncInfo(
                        on_wait=kept,
                        on_update=list(si.on_update) if si else [])
                new_insts.append(inst)
            bb.instructions[:] = new_insts
    return n_split


def build(T, fix=True, do_scan=True, do_proj=True, do_hst=True, do_bias=True, part='full'):
    nc = bass.Bass()

    # -------- inputs (per core; weights identical across cores) --------
    xT_d = nc.declare_dram_parameter("xT", [128, 4 * BC], F32R, isOutput=False)
    w1t_d = nc.declare_dram_parameter("w1t", [128, 4 * H], F32R, isOutput=False)
    w2t_d = nc.declare_dram_parameter("w2t", [128, 8 * H], F32R, isOutput=False)
    b1r_d = nc.declare_dram_parameter("b1r", [1, H], F32R, isOutput=False)
    b2r_d = nc.declare_dram_parameter("b2r", [1, H], F32R, isOutput=False)
    wq_d = nc.declare_dram_parameter("wq", [128, 64 * 512], BF16, isOutput=False)
    biasq_d = nc.declare_dram_parameter("biasq", [1, 8 * 512], BF16, isOutput=False)
    owt_d = nc.declare_dram_parameter("owt", [128, 8 * D_OUT], BF16, isOutput=False)
    obr_d = nc.declare_dram_parameter("obr", [1, D_OUT], BF16, isOutput=False)
    onesr_d = nc.declare_dram_parameter("onesr", [1, BC], F32R, isOutput=False)
    onesb_d = nc.declare_dram_parameter("onesb", [1, 128], BF16, isOutput=False)
    ident_d = nc.declare_dram_parameter("ident", [128, 32], F32, isOutput=False)

    recon_d = nc.declare_dram_parameter("recon", [BC, T, D_OUT], F32, isOutput=True)
    lat_d = nc.declare_dram_parameter("latent", [BC, H], F32, isOutput=True)

    lat_tmp = nc.dram_tensor("lat_tmp", [BC, H], F32)
    hsT_d = nc.dram_tensor("hsT", [128, KT * T * BC], BF16)

    with tile.TileContext(nc) as tc:
        with (
            tc.tile_pool(name="const", bufs=1) as cpool,
            tc.tile_pool(name="work", bufs=2) as wpool,
            tc.tile_pool(name="proj", bufs=3) as ppool,
            tc.tile_pool(name="ps", bufs=2, space="PSUM") as ps,
            tc.tile_pool(name="psx", bufs=1, space="PSUM") as psx,
        ):
            # ---------------- load constants ----------------
            xT = cpool.tile([128, 4 * BC], F32R, tag="xT")
            nc.sync.dma_start(out=xT[:], in_=xT_d[:, :])
            w1t = cpool.tile([128, 4 * H], F32R, tag="w1t")
            nc.sync.dma_start(out=w1t[:], in_=w1t_d[:, :])
            w2t = cpool.tile([128, 8 * H], F32R, tag="w2t")
            nc.sync.dma_start(out=w2t[:], in_=w2t_d[:, :])
            b1r = cpool.tile([1, H], F32R, tag="b1r")
            nc.sync.dma_start(out=b1r[:], in_=b1r_d[:, :])
            b2r = cpool.tile([1, H], F32R, tag="b2r")
            nc.sync.dma_start(out=b2r[:], in_=b2r_d[:, :])
            wq = cpool.tile([128, 64 * 512], BF16, tag="wq")
            nc.sync.dma_start(out=wq[:], in_=wq_d[:, :])
            biasq = cpool.tile([1, 8 * 512], BF16, tag="biasq")
            nc.sync.dma_start(out=biasq[:], in_=biasq_d[:, :])
            owt = cpool.tile([128, 8 * D_OUT], BF16, tag="owt")
            nc.sync.dma_start(out=owt[:], in_=owt_d[:, :])
            obr = cpool.tile([1, D_OUT], BF16, tag="obr")
            nc.sync.dma_start(out=obr[:], in_=obr_d[:, :])
            onesr = cpool.tile([1, BC], F32R, tag="onesr")
            nc.sync.dma_start(out=onesr[:], in_=onesr_d[:, :])
            onesb = cpool.tile([1, 128], BF16, tag="onesb")
            nc.sync.dma_start(out=onesb[:], in_=onesb_d[:, :])
            ident = cpool.tile([128, 32], F32, tag="ident")
            nc.sync.dma_start(out=ident[:], in_=ident_d[:, :])

            # ---------------- FCN (fp32r) ----------------
            # l1 = x @ w1.T + b1  -> [32, 1024] as 2 psum tiles of [32, 512]
            l1sb = wpool.tile([BC, H], F32, tag="l1sb")
            for n in range(2):
                pl = ps.tile([BC, 512], F32, tag="pq0")
                nc.tensor.matmul(pl[:], onesr[:], b1r[0:1, n * 512:(n + 1) * 512],
                                 start=True, stop=False)
                for k in range(4):
                    nc.tensor.matmul(
                        pl[:], xT[:, k * BC:(k + 1) * BC],
                        w1t[:, k * H + n * 512:k * H + (n + 1) * 512],
                        start=False, stop=(k == 3))
                nc.scalar.copy(l1sb[:, n * 512:(n + 1) * 512], pl[:])
            # transpose l1 -> l1T [128, 8*32] fp32r
            l1T = wpool.tile([128, KT * BC], F32R, tag="l1T")
            for k in range(KT):
                pt2 = psx.tile([128, BC], F32, tag=f"pt{k % 4}")
                nc.tensor.transpose(pt2[:], l1sb[:, k * 128:(k + 1) * 128],
                                    ident[0:32, :])
                nc.scalar.copy(l1T[:, k * BC:(k + 1) * BC], pt2[:])
            # l2 = l1 @ w2.T + b2 -> latent [32, 1024]
            latsb = wpool.tile([BC, H], F32, tag="latsb")
            for n in range(2):
                pl2 = ps.tile([BC, 512], F32, tag="pq0")
                nc.tensor.matmul(pl2[:], onesr[:], b2r[0:1, n * 512:(n + 1) * 512],
                                 start=True, stop=False)
                for k in range(KT):
                    nc.tensor.matmul(
                        pl2[:], l1T[:, k * BC:(k + 1) * BC],
                        w2t[:, k * H + n * 512:k * H + (n + 1) * 512],
                        start=False, stop=(k == KT - 1))
                nc.scalar.copy(latsb[:, n * 512:(n + 1) * 512], pl2[:])
            nc.sync.dma_start(out=lat_d[:, :], in_=latsb[:])
            nc.sync.dma_start(out=lat_tmp[:, :], in_=latsb[:])
            # initial state in both layouts via DRAM roundtrip
            hq0 = wpool.tile([128, 256], F32, tag="hq")
            for g in range(4):
                nc.sync.dma_start(
                    out=hq0[g * 32:(g + 1) * 32, :].rearrange(
                        "b (q j) -> b q j", q=2),
                    in_=lat_tmp[:].rearrange(
                        "b (q g j) -> g b q j", q=2, g=4)[g])
            hTa0 = wpool.tile([128, 4 * BC], BF16, tag="hTa")
            hTb0 = wpool.tile([128, 4 * BC], BF16, tag="hTb")
            for k in range(KT):
                dst = hTa0 if k < 4 else hTb0
                nc.gpsimd.dma_start(
                    out=dst[:, (k % 4) * BC:(k % 4 + 1) * BC],
                    in_=lat_tmp[:, k * 128:(k + 1) * 128].rearrange("b p -> p b"))

            # ---------------- GRU scan ----------------
            import os as _os
            _rep = int(_os.environ.get("K_REPEAT", "1"))
            hq_prev, hTa_prev, hTb_prev = hq0, hTa0, hTb0
            for t in range(T * _rep if do_scan else 0):
                t = t % T
                hq_next = wpool.tile([128, 256], F32, tag="hq")
                hTa_next = wpool.tile([128, 4 * BC], BF16, tag="hTa")
                hTb_next = wpool.tile([128, 4 * BC], BF16, tag="hTb")
                for q in range(2):
                    pq = ps.tile([128, 512], F32, tag=f"pq{q}")
                    # bias row for each col group, then K-major interleave
                    if do_bias:
                        for g in range(4):
                            c = 4 * q + g
                            nc.tensor.matmul(
                                pq[g * 32:(g + 1) * 32, :],
                                onesb[0:1, 0:BC],
                                biasq[0:1, c * 512:(c + 1) * 512],
                                start=True, stop=False,
                                tile_position=(0, g * 32),
                                skip_group_check=True)
                    for k in range(KT):
                        for g in range(4):
                            c = 4 * q + g
                            hsrc = hTa_prev if k < 4 else hTb_prev
                            nc.tensor.matmul(
                                pq[g * 32:(g + 1) * 32, :],
                                hsrc[:, (k % 4) * BC:(k % 4 + 1) * BC],
                                wq[:, (k * 8 + c) * 512:(k * 8 + c + 1) * 512],
                                start=(not do_bias and k == 0),
                                stop=(k == KT - 1),
                                tile_position=(0, g * 32),
                                skip_group_check=True)
                    if part == "mm":
                        dump = wpool.tile([128, 512], F32, tag=f"dump{q}")
                        nc.vector.tensor_copy(dump[:], pq[:])
                        continue
                    # gates: pq cols = [r 0:128 | z 128:256 | in 256:384 | hn 384:512]
                    # h' = n*(1-z) + z*h ; z-products computed early (off tail)
                    rz = wpool.tile([128, 256], F32, tag=f"rz{q}")
                    nc.scalar.activation(rz[:], pq[:, 0:256], ACTF.Sigmoid)
                    zh = wpool.tile([128, 128], F32, tag=f"zh{q}")
                    nc.vector.tensor_mul(zh[:], rz[:, 128:256],
                                         hq_prev[:, q * 128:(q + 1) * 128])
                    sz = wpool.tile([128, 128], F32, tag=f"sz{q}")
                    nc.vector.tensor_scalar(sz[:], rz[:, 128:256], -1.0, 1.0,
                                            mybir.AluOpType.mult,
                                            mybir.AluOpType.add)
                    t1 = wpool.tile([128, 128], F32, tag=f"t1{q}")
                    nc.vector.tensor_mul(t1[:], rz[:, 0:128], pq[:, 384:512])
                    t2 = wpool.tile([128, 128], F32, tag=f"t2{q}")
                    nc.vector.tensor_add(t2[:], t1[:], pq[:, 256:384])
                    nt = wpool.tile([128, 128], F32, tag=f"nt{q}")
                    nc.scalar.activation(nt[:], t2[:], ACTF.Tanh)
                    t5 = wpool.tile([128, 128], F32, tag=f"t5{q}")
                    nc.vector.tensor_mul(t5[:], nt[:], sz[:])
                    nc.vector.tensor_add(hq_next[:, q * 128:(q + 1) * 128],
                                         t5[:], zh[:])
                    if part == "mmg":
                        continue
                    # transposes: one psum bank per transpose (HW constraint)
                    for g in range(4):
                        c = 4 * q + g
                        ptg = psx.tile([128, BC], F32, tag=f"pt{g}")
                        nc.tensor.transpose(
                            ptg[:],
                            hq_next[g * 32:(g + 1) * 32, q * 128:(q + 1) * 128],
                            ident[g * 32:(g + 1) * 32, :],
                            tile_position=(g * 32, 0))
                        hdst = hTa_next if q == 0 else hTb_next
                        if g % 2 == 0:
                            nc.scalar.copy(hdst[:, g * BC:(g + 1) * BC], ptg[:])
                        else:
                            nc.vector.tensor_copy(hdst[:, g * BC:(g + 1) * BC],
                                                  ptg[:])
                if do_hst:
                    hsv = hsT_d[:].rearrange("p (k tb) -> p k tb", k=KT)
                    nc.sync.dma_start(
                        out=hsv[:, 0:4, t * BC:(t + 1) * BC],
                        in_=hTa_next[:].rearrange("p (k b) -> p k b", k=4))
                    nc.sync.dma_start(
                        out=hsv[:, 4:8, t * BC:(t + 1) * BC],
                        in_=hTb_next[:].rearrange("p (k b) -> p k b", k=4))
                if part == "full":
                    hq_prev, hTa_prev, hTb_prev = hq_next, hTa_next, hTb_next
                elif part == "mmg":
                    hq_prev = hq_next

            # ---------------- output projection (bf16) ----------------
            n_chunks = (T * BC) // 128 if (do_scan and do_proj) else 0
            for i in range(n_chunks):
                ch = ppool.tile([128, KT * 128], BF16, tag="proj_ch")
                nc.sync.dma_start(
                    out=ch[:].rearrange("p (k tb) -> p k tb", k=KT),
                    in_=hsT_d[:].rearrange("p (k tb) -> p k tb", k=KT)
                    [:, :, i * 128:(i + 1) * 128])
                pp = ps.tile([128, D_OUT], F32, tag="pq1")
                nc.tensor.matmul(pp[:], onesb[:], obr[:],
                                 start=True, stop=False)
                for k in range(KT):
                    nc.tensor.matmul(
                        pp[:], ch[:, k * 128:(k + 1) * 128],
                        owt[:, k * D_OUT:(k + 1) * D_OUT],
                        start=False, stop=(k == KT - 1))
                outc = ppool.tile([128, D_OUT], F32, tag="outc")
                if i % 2 == 0:
                    nc.scalar.copy(outc[:], pp[:])
                else:
                    nc.vector.tensor_copy(outc[:], pp[:])
                nc.sync.dma_start(
                    out=recon_d[0:BC, 4 * i:4 * i + 4, :]
                    .rearrange("b t d -> t b d"),
                    in_=outc[:])

    nc._tc_ref = tc
    if fix:
        fix_multiwaits(nc)
    return nc


_built = {}


def _get_nc(T):
    import os
    key = (T, os.environ.get("K_SCAN", "1"), os.environ.get("K_PROJ", "1"),
           os.environ.get("K_FIX", "1"), os.environ.get("K_HST", "1"),
           os.environ.get("K_BIAS", "1"), os.environ.get("K_PART", "full"),
           os.environ.get("K_REPEAT", "1"))
    if key not in _built:
        _built[key] = build(T, fix=key[3] == "1", do_scan=key[1] == "1",
                            do_proj=key[2] == "1", do_hst=key[4] == "1",
                            do_bias=key[5] == "1", part=key[6])
    return _built[key]


def _prepare_maps(x, fcn_w1, fcn_b1, fcn_w2, fcn_b2, w_ih, b_ih, w_hh, b_hh,
                  out_w, out_b):
    # FCN weights (fp32r), k-major partition layout [128, K*cols]
    w1t = _round_fp32r(
        fcn_w1.T.reshape(4, 128, H).transpose(1, 0, 2).reshape(128, 4 * H))
    w2t = _round_fp32r(
        fcn_w2.T.reshape(8, 128, H).transpose(1, 0, 2).reshape(128, 8 * H))
    b1r = _round_fp32r(fcn_b1.reshape(1, H))
    b2r = _round_fp32r(fcn_b2.reshape(1, H))

    # combined gate weights: Wc rows = [r(1024) z(1024) in(1024) hn(1024)]
    Wc = np.concatenate([w_ih[:2 * H] + w_hh[:2 * H], w_ih[2 * H:],
                         w_hh[2 * H:]], axis=0)          # [4H, H]
    biasc = np.concatenate([b_ih[:2 * H] + b_hh[:2 * H], b_ih[2 * H:],
                            b_hh[2 * H:]])               # [4H]
    # wq[p, (k*8+c)*512 + gate*128 + j] = Wc[gate*1024 + c*128 + j, k*128+p]
    wq = _bf16(Wc.reshape(4, 8, 128, 8, 128)            # [gate, c, j, k, p]
               .transpose(4, 3, 1, 0, 2)                # [p, k, c, gate, j]
               .reshape(128, 64 * 512))
    # biasq[0, c*512 + gate*128 + j] = biasc[gate*1024 + c*128 + j]
    biasq = _bf16(biasc.reshape(4, 8, 128)              # [gate, c, j]
                  .transpose(1, 0, 2)                   # [c, gate, j]
                  .reshape(1, 8 * 512))
    owt = _bf16(out_w.T.reshape(8, 128, D_OUT)
                .transpose(1, 0, 2).reshape(128, 8 * D_OUT))
    obr = _bf16(out_b.reshape(1, D_OUT))
    onesr = _round_fp32r(np.ones((1, BC), np.float32))
    onesb = _bf16(np.ones((1, 128), np.float32))
    ident = np.zeros((128, 32), np.float32)
    for g in range(4):
        ident[g * 32:(g + 1) * 32, :] = np.eye(32, dtype=np.float32)

    maps = []
    for c in range(NC):
        xc = x[c * BC:(c + 1) * BC]                     # [32, 512]
        xT = _round_fp32r(
            xc.T.reshape(4, 128, BC).transpose(1, 0, 2).reshape(128, 4 * BC))
        maps.append({
            "xT": xT, "w1t": w1t, "w2t": w2t, "b1r": b1r, "b2r": b2r,
            "wq": wq, "biasq": biasq, "owt": owt, "obr": obr,
            "onesr": onesr, "onesb": onesb, "ident": ident,
        })
    return maps


def kernel(x, fcn_w1, fcn_b1, fcn_w2, fcn_b2, w_ih, b_ih, w_hh, b_hh,
           out_w, out_b, n_steps, _trace=False, _trace_kwargs=None):
    T = int(n_steps)
    x = np.ascontiguousarray(x, np.float32)
    args = [np.ascontiguousarray(np.asarray(a), np.float32)
            for a in (fcn_w1, fcn_b1, fcn_w2, fcn_b2, w_ih, b_ih, w_hh, b_hh,
                      out_w, out_b)]
    nc = _get_nc(T)
    maps = _prepare_maps(x, *args)
    kw = {}
    if _trace:
        kw = {"trace": True}
        if _trace_kwargs:
            kw.update(_trace_kwargs)
    res = run_bass_kernel_spmd(nc, maps, list(range(NC)), **kw)
    recon = np.concatenate([res.results[c]["recon"] for c in range(NC)], axis=0)
    lat = np.concatenate([res.results[c]["latent"] for c in range(NC)], axis=0)
    kernel._last_results = res
    return recon, lat
